# revision 40
# baseline (speedup 1.0000x reference)
"""Fused attention block (qkv proj + pooled attention + 16-head masked
attention + out proj) for TRN2, batch-parallel across 8 NeuronCores.

Key optimizations vs the straightforward version:
  - Key/value compaction: masked keys (~50%) are dropped on the host by
    gathering kept columns of x^T into xcT [D, 640] (padded with CLS,
    pad slots get a -80 exp bias). V-proj, head dots, exp, and attn@V
    all shrink by ~3/8. The pooled attention is unmasked and keeps the
    full K.
  - attn@V is "flipped": stationary = exp(dots) [jc, i-block], moving =
    per-head V plus a ones column [jc, 65] in bf16 (bf16 keeps 1
    cycle/row at a 65-wide moving operand). Output lands i-major with
    the softmax denominator in the 65th column -> per-partition
    normalization on DVE, then a PE transpose to feature-major aoT.
  - Masked queries are zeroed via a per-partition qkeep multiply and
    fixed at the out-projection by a single K=2 rank-2 correction
    matmul (ones x bout + (1-m)/N x ybar), ybar computed on the host.
  - Software pipelining: QK-proj matmuls of pair F+1 are interleaved
    into the ACT-paced dots/exp stretch of pair F (separate PSUM pools
    so slot reuse cannot serialize them); attn@V of pair F runs on the
    resident exp tiles right after. Transposes are deferred into the
    pooled-attention phase; out-projection is d-split with one bf16
    wout stream.
  - PSUM start-bit semantics: `start` zeroes a whole 2KB bank region,
    so per bank only the first matmul sets it.
"""
import os
import sys
from contextlib import ExitStack

sys.path.insert(0, "/opt/trn_rl_repo")

import numpy as np

import concourse.bass as bass
import concourse.mybir as mybir
import concourse.tile as tile
from concourse import bacc, bass_utils

F32 = mybir.dt.float32
F32R = mybir.dt.float32r
BF16 = mybir.dt.bfloat16
EXP = mybir.ActivationFunctionType.Exp

B = 8
N = 1024          # sequence (after CLS pad)
D = 1024          # model dim
H = 16
DH = 64
NT = N // 128     # 8 row tiles
JC = 640          # compacted key count (kept keys padded)
JT = JC // 128    # 5 compacted j tiles
SCALE_H = DH ** -0.5     # 1/8
SCALE_P = D ** -0.5      # 1/32
NEG = -80.0

_CACHED = {}


def _body(nc, tc, es, t_in, t_out):
    pool = lambda **kw: es.enter_context(tc.tile_pool(**kw))
    xtp = pool(name="xt", bufs=2)
    qkp = pool(name="qk", bufs=16)
    kcp = pool(name="kc", bufs=8)
    vpp = pool(name="vp", bufs=JT)
    aonp = pool(name="aon", bufs=64)
    onep = pool(name="one", bufs=1)
    smallp = pool(name="small", bufs=8)

    # ---- constants ----
    cmask_t = onep.tile([128, JT], F32, name="cmask_t", tag="cmask_t")
    nc.gpsimd.dma_start(out=cmask_t, in_=t_in["cmaskTc"])
    qkT_t = onep.tile([128, NT], F32, name="qkT_t", tag="qkT_t")
    nc.gpsimd.dma_start(out=qkT_t, in_=t_in["qkeepT"])
    fixl = onep.tile([2, N], BF16, name="fixl", tag="fixl")
    nc.gpsimd.dma_start(out=fixl, in_=t_in["fixl_in"])
    fixr = onep.tile([2, N], BF16, name="fixr", tag="fixr")
    nc.gpsimd.dma_start(out=fixr, in_=t_in["fixr_in"])
    ident = onep.tile([128, 128], BF16, name="ident", tag="ident")
    nc.gpsimd.dma_start(out=ident, in_=t_in["ident_in"])

    xts = []

    v_tiles = []
    for t in range(JT):
        vt = vpp.tile([128, 65 * H], BF16, tag="v", name=f"v{t}")
        v3 = vt.rearrange("p (h d) -> p h d", d=65)
        nc.vector.memset(v3[:, :, 64:65], 1.0)
        v_tiles.append(vt)

    kc_tiles = []
    qk_tiles = [None] * (2 * NT)
    ao_tiles = [None] * NT
    aon_tiles = [[None] * NT for _ in range(NT)]

    def phase_vkc(xcp, wvp, wk2p, vpsp, kcpsp):
        # xcT (then xT) on the Activation queue; batched weights on SP.
        xcts = []
        for t in range(NT):
            xct = xcp.tile([128, JC], F32R, tag="xc", name=f"xct{t}")
            nc.scalar.dma_start(out=xct,
                                in_=t_in["xcT"][t * 128:(t + 1) * 128, :])
            xcts.append(xct)
        xt4 = t_in["xT"].rearrange("(kt p) d -> p kt d", p=128)
        xbig = []
        for g in range(2):
            xb = xtp.tile([128, 4, N], F32R, tag="xt", name=f"xt{g}")
            nc.scalar.dma_start(out=xb, in_=xt4[:, g * 4:(g + 1) * 4, :])
            xbig.append(xb)
        for kt in range(NT):
            xts.append(xbig[kt // 4][:, kt % 4, :])
        # wv batches sized so the V kt-loop is never starved
        wv4 = t_in["wv_in"].rearrange("(kt p) d -> p kt d", p=128)
        wvs = {}
        for c, groups in ((0, [(0, 2), (2, 2), (4, 4)]), (1, [(0, 4), (4, 4)])):
            for g0, glen in groups:
                w = wvp.tile([128, 4, 512], F32R, tag="wv",
                             name=f"w_v{c}{g0}")
                nc.sync.dma_start(
                    out=w[:, 0:glen, :],
                    in_=wv4[:, g0:g0 + glen, c * 512:(c + 1) * 512])
                for k in range(glen):
                    wvs[c * 8 + g0 + k] = w[:, k, :]
        # V projection (position-major, bf16 + ones col)
        for c in range(2):
            pss = [vpsp.tile([128, 512], F32, tag="vps", name=f"ps_v{c}{j}")
                   for j in range(JT)]
            for kt in range(NT):
                for jcb in range(JT):
                    nc.tensor.matmul(
                        pss[jcb][:],
                        xcts[kt][:, jcb * 128:(jcb + 1) * 128],
                        wvs[c * 8 + kt],
                        start=(kt == 0), stop=(kt == NT - 1),
                    )
            for jcb in range(JT):
                v3 = v_tiles[jcb].rearrange("p (h d) -> p h d", d=65)
                src = pss[jcb].rearrange("p (h d) -> p h d", d=64)
                nc.vector.tensor_copy(v3[:, c * 8:(c + 1) * 8, 0:64], src)
        # compact K projection (f-major kc tiles [128, JC]);
        # k-half weights batched: one [128, 8*128] load per kfb
        wq4 = t_in["wqkp"].rearrange("(kt p) d -> p kt d", p=128)
        for kfb in range(NT):
            wk = wk2p.tile([128, NT, 128], F32R, tag="wk2", name=f"w_k2{kfb}")
            nc.sync.dma_start(
                out=wk, in_=wq4[:, :, kfb * 256 + 128:kfb * 256 + 256])
            ps = kcpsp.tile([128, N], F32, tag="kcps", name=f"ps_kc{kfb}")
            for kt in range(NT):
                for c2 in range(2):
                    nc.tensor.matmul(
                        ps[:, c2 * 512:c2 * 512 + 320],
                        wk[:, kt, :],
                        xcts[kt][:, c2 * 320:(c2 + 1) * 320],
                        start=(kt == 0), stop=(kt == NT - 1),
                    )
            kc = kcp.tile([128, JC], F32R, tag="kc", name=f"kc{kfb}")
            nc.vector.tensor_copy(kc[:, 0:320], ps[:, 0:320])
            nc.vector.tensor_copy(kc[:, 320:640], ps[:, 512:832])
            kc_tiles.append(kc)
            if kfb == 0 and "dbg_kc" in t_out:
                nc.gpsimd.dma_start(out=t_out["dbg_kc"], in_=kc)
                nc.gpsimd.dma_start(out=t_out["dbg_v"], in_=v_tiles[0])

    def head_loop(ptp, wqp, projp, dpp, Ppp, pep):
        wq4 = t_in["wqkp"].rearrange("(kt p) d -> p kt d", p=128)

        def load_w(fpair, eng):
            parts = []
            for g in range(2):
                w = wqp.tile([128, 4, 256], F32R, tag="wq",
                             name=f"wq{fpair}{g}")
                eng.dma_start(
                    out=w, in_=wq4[:, g * 4:(g + 1) * 4,
                                   fpair * 256:(fpair + 1) * 256])
                parts.append(w)
            return parts

        def proj_ops(fpair, ws):
            """Flat op list: 4 sequential [128,512] chunk accumulations."""
            ops = []
            for half in (0, 1):
                ft = fpair + half * NT
                qk_tiles[ft] = qkp.tile([128, N], F32R, tag="qk",
                                        name=f"qk{ft}")
            for half in (0, 1):
                ft = fpair + half * NT
                for c in (0, 1):
                    ps = projp.tile([128, 512], F32, tag="pj",
                                    name=f"pj{ft}{c}")
                    for kt in range(NT):
                        ops.append(lambda ps=ps, kt=kt, half=half, c=c:
                                   nc.tensor.matmul(
                                       ps[:],
                                       ws[kt // 4][:, kt % 4,
                                                   half * 128:(half + 1) * 128],
                                       xts[kt][:, c * 512:(c + 1) * 512],
                                       start=(kt == 0), stop=(kt == NT - 1)))
                    ops.append(lambda ps=ps, ft=ft, c=c:
                               nc.vector.tensor_copy(
                                   qk_tiles[ft][:, c * 512:(c + 1) * 512], ps))
            return ops

        def pooled_early(it):
            """Matmul ops for pooled attention tile `it` into two proj-pool
            chunks; returns (ops, finish) where finish() emits exp+norm+store.
            """
            chunks = [projp.tile([128, 512], F32, tag="pj",
                                 name=f"pool{it}{c}") for c in range(2)]
            ops = []
            for c in range(2):
                for ft in range(NT):
                    ops.append(lambda c=c, ft=ft: nc.tensor.matmul(
                        chunks[c][:],
                        qk_tiles[ft][:, it * 128:(it + 1) * 128],
                        qk_tiles[NT + ft][:, c * 512:(c + 1) * 512],
                        start=(ft == 0), stop=(ft == NT - 1)))

            def finish():
                pe = pep.tile([128, N], F32, tag="pe", name=f"pe{it}")
                s0 = smallp.tile([128, 2], F32, tag="sm2", bufs=2,
                                 name=f"s2{it}")
                for c in range(2):
                    nc.scalar.activation(
                        pe[:, c * 512:(c + 1) * 512], chunks[c], EXP,
                        scale=SCALE_P, accum_out=s0[:, c:c + 1])
                sums = smallp.tile([128, 1], F32, tag="sm", name=f"sums{it}")
                nc.vector.tensor_add(sums, s0[:, 0:1], s0[:, 1:2])
                rec = smallp.tile([128, 1], F32, tag="rc", name=f"rcp{it}")
                nc.vector.reciprocal(rec, sums)
                nc.vector.tensor_scalar_mul(pe, pe, rec)
                nc.gpsimd.dma_start(
                    out=t_out["attn"][it * 128:(it + 1) * 128, :], in_=pe)
            return ops, finish

        ws = load_w(0, nc.gpsimd)
        pend = proj_ops(0, ws)
        while pend:
            pend.pop(0)()

        for hp in range(NT):
            finishers = []
            if hp + 1 < NT:
                ws = load_w(hp + 1, nc.sync)
                pend = proj_ops(hp + 1, ws)
            else:
                pend = []
                for it_e in (0, 1):
                    ops, fin = pooled_early(it_e)
                    pend += ops
                    finishers.append(fin)

            # dots + exp stretch, proj(F+1) matmuls woven in
            pts = {}
            for s in range(2 * JT):
                jt, u = divmod(s, 2)
                dp = dpp.tile([128, N], F32, tag="dp", name=f"dp{hp}{u}{jt}")
                off = u * 64
                for c in range(2):
                    nc.tensor.matmul(
                        dp[:, c * 512:(c + 1) * 512],
                        kc_tiles[hp][off:off + 64, jt * 128:(jt + 1) * 128],
                        qk_tiles[hp][off:off + 64, c * 512:(c + 1) * 512],
                        start=True, stop=True,
                    )
                pt = ptp.tile([128, N], BF16, tag="pt", name=f"pt{hp}{u}{jt}")
                nc.scalar.activation(
                    pt, dp, EXP, bias=cmask_t[:, jt:jt + 1], scale=SCALE_H)
                pts[(u, jt)] = pt
                if hp == 0 and s == 0 and "dbg_pt" in t_out:
                    nc.gpsimd.dma_start(out=t_out["dbg_pt"], in_=pt)
                for _ in range(4):
                    if pend:
                        pend.pop(0)()
            while pend:
                pend.pop(0)()
            for fin in finishers:
                fin()

            # attn@V on resident pt tiles + per-partition normalization
            for it in range(NT):
                P = Ppp.tile([128, 512], F32, tag="P", name=f"P{hp}{it}")
                for jt in range(JT):
                    for u in (0, 1):
                        h = 2 * hp + u
                        nc.tensor.matmul(
                            P[:, u * 65:u * 65 + 65],
                            pts[(u, jt)][:, it * 128:(it + 1) * 128],
                            v_tiles[jt][:, h * 65:(h + 1) * 65],
                            start=(jt == 0 and u == 0),
                            stop=(jt == JT - 1 and u == 1),
                        )
                if hp == 0 and it == 0 and "dbg_P" in t_out:
                    dbgp = smallp.tile([128, 512], F32, tag="dbgp", bufs=1,
                                       name="dbgp")
                    nc.vector.tensor_copy(dbgp, P)
                    nc.gpsimd.dma_start(out=t_out["dbg_P"], in_=dbgp)
                rec = smallp.tile([128, 2], F32, tag="rec", bufs=3,
                                  name=f"rec{hp}{it}")
                nc.vector.reciprocal(rec, P[:, 64:130:65])
                rq = smallp.tile([128, 2], F32, tag="rq", bufs=3,
                                 name=f"rq{hp}{it}")
                nc.vector.tensor_scalar_mul(rq, rec, qkT_t[:, it:it + 1])
                aon = aonp.tile([128, 128], BF16, tag="aon",
                                name=f"aon{hp}{it}")
                for u in (0, 1):
                    nc.vector.tensor_scalar_mul(
                        aon[:, u * 64:(u + 1) * 64],
                        P[:, u * 65:u * 65 + 64],
                        rq[:, u:u + 1])
                aon_tiles[hp][it] = aon

        # remaining pooled-attention tiles, PE-dense over the exp drain
        for it_l in range(2, NT):
            ops, fin = pooled_early(it_l)
            for op in ops:
                op()
            fin()

    def phase_tr(trp, aop):
        for hp in range(NT):
            tr = trp.tile([128, N], BF16, tag="tr", name=f"tr{hp}")
            for k in range(NT):
                nc.tensor.matmul(
                    tr[:, k * 128:(k + 1) * 128],
                    aon_tiles[hp][k][:], ident[:], is_transpose=True)
            ao = aop.tile([128, N], BF16, tag="ao", name=f"ao{hp}")
            nc.vector.tensor_copy(ao, tr)
            ao_tiles[hp] = ao
            if hp == 0 and "dbg_ao" in t_out:
                nc.gpsimd.dma_start(out=t_out["dbg_ao"], in_=ao)

    def phase_out(wop, otp, opsp):
        for dh in range(2):
            pss = [opsp.tile([128, 512], F32, tag="ops", name=f"ps_o{dh}{i}")
                   for i in range(NT)]
            for ft in range(NT):
                w = wop.tile([128, 512], BF16, tag="wo", name="w_o")
                nc.scalar.dma_start(
                    out=w, in_=t_in["wout"][ft * 128:(ft + 1) * 128,
                                            dh * 512:(dh + 1) * 512])
                for it in range(NT):
                    nc.tensor.matmul(
                        pss[it][:],
                        ao_tiles[ft][:, it * 128:(it + 1) * 128],
                        w[:],
                        start=(ft == 0), stop=False,
                    )
            for it in range(NT):
                nc.tensor.matmul(
                    pss[it][:],
                    fixl[:, it * 128:(it + 1) * 128],
                    fixr[:, dh * 512:(dh + 1) * 512],
                    start=False, stop=True,
                )
                ot = otp.tile([128, 512], F32, tag="ot", bufs=8,
                              name=f"ot{dh}{it}")
                nc.vector.tensor_copy(ot, pss[it])
                eng = nc.sync if it % 2 == 0 else nc.scalar
                eng.dma_start(
                    out=t_out["out"][it * 128:(it + 1) * 128,
                                     dh * 512:(dh + 1) * 512],
                    in_=ot)

    with (
        tc.tile_pool(name="xc", bufs=8) as xcp,
        tc.tile_pool(name="wv", bufs=3) as wvp,
        tc.tile_pool(name="wk2", bufs=2) as wk2p,
        tc.tile_pool(name="vps", bufs=JT, space="PSUM") as vpsp,
        tc.tile_pool(name="kcps", bufs=1, space="PSUM") as kcpsp,
    ):
        phase_vkc(xcp, wvp, wk2p, vpsp, kcpsp)

    with (
        tc.tile_pool(name="pe", bufs=2) as pep,
        tc.tile_pool(name="ao", bufs=8) as aop,
    ):
        with (
            tc.tile_pool(name="pt", bufs=10) as ptp,
            tc.tile_pool(name="wq", bufs=3) as wqp,
            tc.tile_pool(name="pj", bufs=2, space="PSUM") as projp,
            tc.tile_pool(name="dp", bufs=2, space="PSUM") as dpp,
            tc.tile_pool(name="Pp", bufs=2, space="PSUM") as Ppp,
        ):
            head_loop(ptp, wqp, projp, dpp, Ppp, pep)

        with tc.tile_pool(name="tr", bufs=2, space="PSUM") as trp:
            phase_tr(trp, aop)

        with (
            tc.tile_pool(name="wo", bufs=3) as wop,
            tc.tile_pool(name="ott", bufs=4) as otp,
            tc.tile_pool(name="ops", bufs=8, space="PSUM") as opsp,
        ):
            phase_out(wop, otp, opsp)


def build_nc():
    nc = bacc.Bacc("TRN2", target_bir_lowering=False, debug=False,
                   num_devices=8)
    t_in = {
        "xT": nc.dram_tensor("xT", [D, N], F32R, kind="ExternalInput").ap(),
        "xcT": nc.dram_tensor("xcT", [D, JC], F32R, kind="ExternalInput").ap(),
        "wqkp": nc.dram_tensor("wqkp", [D, 2 * D], F32R,
                               kind="ExternalInput").ap(),
        "wv_in": nc.dram_tensor("wv_in", [D, D], F32R,
                                kind="ExternalInput").ap(),
        "wout": nc.dram_tensor("wout", [D, D], BF16,
                               kind="ExternalInput").ap(),
        "cmaskTc": nc.dram_tensor("cmaskTc", [128, JT], F32,
                                  kind="ExternalInput").ap(),
        "qkeepT": nc.dram_tensor("qkeepT", [128, NT], F32,
                                 kind="ExternalInput").ap(),
        "fixl_in": nc.dram_tensor("fixl_in", [2, N], BF16,
                                  kind="ExternalInput").ap(),
        "fixr_in": nc.dram_tensor("fixr_in", [2, N], BF16,
                                  kind="ExternalInput").ap(),
        "ident_in": nc.dram_tensor("ident_in", [128, 128], BF16,
                                   kind="ExternalInput").ap(),
    }
    t_out = {
        "out": nc.dram_tensor("out", [N, D], F32, kind="ExternalOutput").ap(),
        "attn": nc.dram_tensor("attn", [N, N], F32,
                               kind="ExternalOutput").ap(),
    }
    if os.environ.get("ATTN_DEBUG"):
        t_out["dbg_kc"] = nc.dram_tensor(
            "dbg_kc", [128, JC], F32R, kind="ExternalOutput").ap()
        t_out["dbg_v"] = nc.dram_tensor(
            "dbg_v", [128, 65 * H], BF16, kind="ExternalOutput").ap()
        t_out["dbg_pt"] = nc.dram_tensor(
            "dbg_pt", [128, N], BF16, kind="ExternalOutput").ap()
        t_out["dbg_P"] = nc.dram_tensor(
            "dbg_P", [128, 512], F32, kind="ExternalOutput").ap()
        t_out["dbg_ao"] = nc.dram_tensor(
            "dbg_ao", [128, N], BF16, kind="ExternalOutput").ap()
    with tile.TileContext(
            nc, trace_sim=bool(os.environ.get('ATTN_TRACE_SIM'))) as tc:
        with ExitStack() as es:
            _body(nc, tc, es, t_in, t_out)
    nc.compile()
    return nc


def _bf16(a):
    import ml_dtypes
    return np.asarray(a, np.float32).astype(ml_dtypes.bfloat16)


def _host_prep(x, mask, w_qkv, w_out, b_out):
    wq3 = w_qkv.reshape(D, 3, NT, 128)
    wqkp = np.ascontiguousarray(
        np.stack([wq3[:, 0], wq3[:, 1]], axis=2).reshape(D, 2 * D))
    wv = np.ascontiguousarray(w_qkv[:, 2 * D:])
    wout_b = _bf16(w_out)
    ident = _bf16(np.eye(128, dtype=np.float32))
    in_maps = []
    for b in range(B):
        m = np.concatenate([[True], mask[b]])               # [N]
        keep = np.nonzero(m)[0]
        nk = len(keep)
        assert nk <= JC, f"keep count {nk} exceeds JC={JC}"
        idx = np.concatenate([keep, np.zeros(JC - nk, np.int64)])
        cm_c = np.where(np.arange(JC) < nk, 0.0, NEG).astype(np.float32)
        xTb = np.ascontiguousarray(x[b].T)                  # [D, N]
        xcT = np.ascontiguousarray(xTb[:, idx])             # [D, JC]
        mf = m.astype(np.float32)
        ybar = (x[b].mean(axis=0) @ wv) @ w_out             # [D]
        in_maps.append({
            "xT": xTb,
            "xcT": xcT,
            "wqkp": wqkp,
            "wv_in": wv,
            "wout": wout_b,
            "cmaskTc": np.ascontiguousarray(cm_c.reshape(JT, 128).T),
            "qkeepT": np.ascontiguousarray(mf.reshape(NT, 128).T),
            "fixl_in": _bf16(np.stack([np.ones(N, np.float32), 1.0 - mf])),
            "fixr_in": _bf16(np.stack([b_out, ybar])),
            "ident_in": ident,
        })
    return in_maps


def kernel(x, mask, w_qkv, w_out, b_out, **run_kw):
    if "nc" not in _CACHED:
        _CACHED["nc"] = build_nc()
    nc = _CACHED["nc"]
    in_maps = _host_prep(
        np.asarray(x, np.float32), np.asarray(mask),
        np.asarray(w_qkv, np.float32), np.asarray(w_out, np.float32),
        np.asarray(b_out, np.float32))
    try:
        res = bass_utils.run_bass_kernel_spmd(
            nc, in_maps, core_ids=list(range(B)), **run_kw)
    except Exception:
        # transient NRT device wedge: retry once
        res = bass_utils.run_bass_kernel_spmd(
            nc, in_maps, core_ids=list(range(B)), **run_kw)
    out = np.stack([res.results[b]["out"] for b in range(B)])
    attn_ = np.stack([res.results[b]["attn"] for b in range(B)])
    _CACHED["last_results"] = res
    return out, attn_


# revision 59
# speedup vs baseline: 1.1016x; 1.1016x over previous
"""Fused attention block (qkv proj + pooled attention + 16-head masked
attention + out proj) for TRN2, batch-parallel across 8 NeuronCores.

Key optimizations vs the straightforward version:
  - Key/value compaction: masked keys (~50%) are dropped on the host by
    gathering kept columns of x^T into xcT [D, 640] (padded with CLS,
    pad slots get a -80 exp bias). V-proj, head dots, exp, and attn@V
    all shrink by ~3/8. The pooled attention is unmasked and keeps the
    full K.
  - attn@V is "flipped": stationary = exp(dots) [jc, i-block], moving =
    per-head V plus a ones column [jc, 65] in bf16 (bf16 keeps 1
    cycle/row at a 65-wide moving operand). Output lands i-major with
    the softmax denominator in the 65th column -> per-partition
    normalization on DVE, then a PE transpose to feature-major aoT.
  - Masked queries are zeroed via a per-partition qkeep multiply and
    fixed at the out-projection by a single K=2 rank-2 correction
    matmul (ones x bout + (1-m)/N x ybar), ybar computed on the host.
  - Software pipelining: QK-proj matmuls of pair F+1 are interleaved
    into the ACT-paced dots/exp stretch of pair F (separate PSUM pools
    so slot reuse cannot serialize them); attn@V of pair F runs on the
    resident exp tiles right after. Transposes are deferred into the
    pooled-attention phase; out-projection is d-split with one bf16
    wout stream.
  - PSUM start-bit semantics: `start` zeroes a whole 2KB bank region,
    so per bank only the first matmul sets it.
"""
import os
import sys
from contextlib import ExitStack

sys.path.insert(0, "/opt/trn_rl_repo")

import numpy as np

import concourse.bass as bass
import concourse.mybir as mybir
import concourse.tile as tile
from concourse import bacc, bass_utils

F32 = mybir.dt.float32
F32R = mybir.dt.float32r
BF16 = mybir.dt.bfloat16
EXP = mybir.ActivationFunctionType.Exp

B = 8
N = 1024          # sequence (after CLS pad)
D = 1024          # model dim
H = 16
DH = 64
NT = N // 128     # 8 row tiles
JC = 640          # compacted key count (kept keys padded)
JT = JC // 128    # 5 compacted j tiles
SCALE_H = DH ** -0.5     # 1/8
SCALE_P = D ** -0.5      # 1/32
NEG = -80.0

_CACHED = {}


def _body(nc, tc, es, t_in, t_out):
    pool = lambda **kw: es.enter_context(tc.tile_pool(**kw))
    xtp = pool(name="xt", bufs=2)
    qkp = pool(name="qk", bufs=16)
    kcp = pool(name="kc", bufs=8)
    vpp = pool(name="vp", bufs=JT)
    onep = pool(name="one", bufs=1)
    smallp = pool(name="small", bufs=8)

    # ---- constants ----
    cmask_t = onep.tile([128, JT], F32, name="cmask_t", tag="cmask_t")
    nc.gpsimd.dma_start(out=cmask_t, in_=t_in["cmaskTc"])
    fixl = onep.tile([1, JC], BF16, name="fixl", tag="fixl")
    nc.gpsimd.dma_start(out=fixl, in_=t_in["fixl_in"])
    fixr = onep.tile([1, N], BF16, name="fixr", tag="fixr")
    nc.gpsimd.dma_start(out=fixr, in_=t_in["fixr_in"])
    ident = onep.tile([128, 128], BF16, name="ident", tag="ident")
    nc.gpsimd.dma_start(out=ident, in_=t_in["ident_in"])

    xts = []

    v_tiles = []
    for t in range(JT):
        vt = vpp.tile([128, 65 * H], BF16, tag="v", name=f"v{t}")
        v3 = vt.rearrange("p (h d) -> p h d", d=65)
        nc.vector.memset(v3[:, :, 64:65], 1.0)
        v_tiles.append(vt)

    kc_tiles = []
    qc_tiles = []
    qk_tiles = [None] * (2 * NT)
    ao_tiles = [None] * NT
    aon_tiles = [[None] * JT for _ in range(NT)]

    def phase_vkc(xcp, wvp, wk2p, es_vkc):
        # xcT (then xT) on the Activation queue; batched weights on SP.
        xcts = []
        for t in range(NT):
            xct = xcp.tile([128, JC], F32R, tag="xc", name=f"xct{t}")
            nc.scalar.dma_start(out=xct,
                                in_=t_in["xcT"][t * 128:(t + 1) * 128, :])
            xcts.append(xct)
        xt4 = t_in["xT"].rearrange("(kt p) d -> p kt d", p=128)
        xbig = []
        for g in range(2):
            xb = xtp.tile([128, 4, N], F32R, tag="xt", name=f"xt{g}")
            nc.scalar.dma_start(out=xb, in_=xt4[:, g * 4:(g + 1) * 4, :])
            xbig.append(xb)
        for kt in range(NT):
            xts.append(xbig[kt // 4][:, kt % 4, :])
        # wv batches sized so the V kt-loop is never starved
        wv4 = t_in["wv_in"].rearrange("(kt p) d -> p kt d", p=128)
        wvs = {}
        for c in range(2):
            for g0 in range(0, NT, 2):
                w = wvp.tile([128, 2, 512], F32R, tag="wv",
                             name=f"w_v{c}{g0}")
                nc.sync.dma_start(
                    out=w, in_=wv4[:, g0:g0 + 2, c * 512:(c + 1) * 512])
                for k in range(2):
                    wvs[c * 8 + g0 + k] = w[:, k, :]
        # V projection (position-major, bf16 + ones col)
        with tc.tile_pool(name="vps", bufs=JT, space="PSUM") as vpsp:
            for c in range(2):
                pss = [vpsp.tile([128, 512], F32, tag="vps",
                                 name=f"ps_v{c}{j}") for j in range(JT)]
                for kt in range(NT):
                    for jcb in range(JT):
                        nc.tensor.matmul(
                            pss[jcb][:],
                            xcts[kt][:, jcb * 128:(jcb + 1) * 128],
                            wvs[c * 8 + kt],
                            start=(kt == 0), stop=(kt == NT - 1),
                        )
                for jcb in range(JT):
                    v3 = v_tiles[jcb].rearrange("p (h d) -> p h d", d=65)
                    src = pss[jcb].rearrange("p (h d) -> p h d", d=64)
                    nc.vector.tensor_copy(v3[:, c * 8:(c + 1) * 8, 0:64], src)
        # compact Q and K projections (f-major [128, JC] tiles);
        # full q|k weight pairs batched: one [128, 8*256] load per kfb
        wq4 = t_in["wqkp"].rearrange("(kt p) d -> p kt d", p=128)
        kcpsp = es_vkc.enter_context(
            tc.tile_pool(name="kcps", bufs=1, space="PSUM"))
        for kfb in range(NT):
            wk = wk2p.tile([128, NT, 256], F32R, tag="wk2", name=f"w_k2{kfb}")
            nc.sync.dma_start(
                out=wk, in_=wq4[:, :, kfb * 256:(kfb + 1) * 256])
            psq = kcpsp.tile([128, N], F32, tag="qcps", bufs=1,
                             name=f"ps_qc{kfb}")
            psk = kcpsp.tile([128, N], F32, tag="kcps", bufs=1,
                             name=f"ps_kc{kfb}")
            for kt in range(NT):
                for c2 in range(2):
                    nc.tensor.matmul(
                        psq[:, c2 * 512:c2 * 512 + 320],
                        wk[:, kt, 0:128],
                        xcts[kt][:, c2 * 320:(c2 + 1) * 320],
                        start=(kt == 0), stop=(kt == NT - 1),
                    )
                    nc.tensor.matmul(
                        psk[:, c2 * 512:c2 * 512 + 320],
                        wk[:, kt, 128:256],
                        xcts[kt][:, c2 * 320:(c2 + 1) * 320],
                        start=(kt == 0), stop=(kt == NT - 1),
                    )
            qc = kcp.tile([128, JC], F32R, tag="qc", name=f"qc{kfb}")
            nc.vector.tensor_copy(qc[:, 0:320], psq[:, 0:320])
            nc.vector.tensor_copy(qc[:, 320:640], psq[:, 512:832])
            qc_tiles.append(qc)
            kc = kcp.tile([128, JC], F32R, tag="kc", name=f"kc{kfb}")
            nc.vector.tensor_copy(kc[:, 0:320], psk[:, 0:320])
            nc.vector.tensor_copy(kc[:, 320:640], psk[:, 512:832])
            kc_tiles.append(kc)
            if kfb == 0 and "dbg_kc" in t_out:
                nc.gpsimd.dma_start(out=t_out["dbg_kc"], in_=kc)
                nc.gpsimd.dma_start(out=t_out["dbg_v"], in_=v_tiles[0])

    def head_loop(ptp, wqp, projp, dpp, Ppp, pep, aonp):
        wq4 = t_in["wqkp"].rearrange("(kt p) d -> p kt d", p=128)

        def load_w(fpair, eng):
            parts = []
            for g in range(2):
                w = wqp.tile([128, 4, 256], F32R, tag="wq",
                             name=f"wq{fpair}{g}")
                eng.dma_start(
                    out=w, in_=wq4[:, g * 4:(g + 1) * 4,
                                   fpair * 256:(fpair + 1) * 256])
                parts.append(w)
            return parts

        def proj_ops(fpair, ws):
            """Flat op list: 4 sequential [128,512] chunk accumulations."""
            ops = []
            for half in (0, 1):
                ft = fpair + half * NT
                qk_tiles[ft] = qkp.tile([128, N], F32R, tag="qk",
                                        name=f"qk{ft}")
            for half in (0, 1):
                ft = fpair + half * NT
                for c in (0, 1):
                    ps = projp.tile([128, 512], F32, tag="pj",
                                    name=f"pj{ft}{c}")
                    for kt in range(NT):
                        ops.append(lambda ps=ps, kt=kt, half=half, c=c:
                                   nc.tensor.matmul(
                                       ps[:],
                                       ws[kt // 4][:, kt % 4,
                                                   half * 128:(half + 1) * 128],
                                       xts[kt][:, c * 512:(c + 1) * 512],
                                       start=(kt == 0), stop=(kt == NT - 1)))
                    ops.append(lambda ps=ps, ft=ft, c=c:
                               nc.vector.tensor_copy(
                                   qk_tiles[ft][:, c * 512:(c + 1) * 512], ps))
            return ops

        def pooled_early(it):
            """Matmul ops for pooled attention tile `it` into two proj-pool
            chunks; returns (ops, finish) where finish() emits exp+norm+store.
            """
            chunks = [projp.tile([128, 512], F32, tag="pj",
                                 name=f"pool{it}{c}") for c in range(2)]
            ops = []
            for c in range(2):
                for ft in range(NT):
                    ops.append(lambda c=c, ft=ft: nc.tensor.matmul(
                        chunks[c][:],
                        qk_tiles[ft][:, it * 128:(it + 1) * 128],
                        qk_tiles[NT + ft][:, c * 512:(c + 1) * 512],
                        start=(ft == 0), stop=(ft == NT - 1)))

            def finish():
                pe = pep.tile([128, N], F32, tag="pe", name=f"pe{it}")
                s0 = smallp.tile([128, 2], F32, tag="sm2", bufs=2,
                                 name=f"s2{it}")
                for c in range(2):
                    nc.scalar.activation(
                        pe[:, c * 512:(c + 1) * 512], chunks[c], EXP,
                        scale=SCALE_P, accum_out=s0[:, c:c + 1])
                sums = smallp.tile([128, 1], F32, tag="sm", name=f"sums{it}")
                nc.vector.tensor_add(sums, s0[:, 0:1], s0[:, 1:2])
                rec = smallp.tile([128, 1], F32, tag="rc", name=f"rcp{it}")
                nc.vector.reciprocal(rec, sums)
                nc.vector.tensor_scalar_mul(pe, pe, rec)
                nc.gpsimd.dma_start(
                    out=t_out["attn"][it * 128:(it + 1) * 128, :], in_=pe)
            return ops, finish

        ws = load_w(0, nc.gpsimd)
        pend = proj_ops(0, ws)
        while pend:
            pend.pop(0)()

        for hp in range(NT):
            finishers = []
            if hp + 1 < NT:
                ws = load_w(hp + 1, nc.sync)
                pend = proj_ops(hp + 1, ws)
            else:
                pend = []
                for it_e in (0, 1):
                    ops, fin = pooled_early(it_e)
                    pend += ops
                    finishers.append(fin)

            # dots + exp stretch, proj(F+1) matmuls woven in
            pts = {}
            for s in range(2 * JT):
                jt, u = divmod(s, 2)
                dp = dpp.tile([128, N], F32, tag="dp", name=f"dp{hp}{u}{jt}")
                off = u * 64
                for c in range(2):
                    nc.tensor.matmul(
                        dp[:, c * 512:c * 512 + 320],
                        kc_tiles[hp][off:off + 64, jt * 128:(jt + 1) * 128],
                        qc_tiles[hp][off:off + 64, c * 320:(c + 1) * 320],
                        start=True, stop=True,
                    )
                pt = ptp.tile([128, JC], BF16, tag="pt", name=f"pt{hp}{u}{jt}")
                dp3 = dp.rearrange("p (g x) -> p g x", x=512)
                pt3 = pt.rearrange("p (g x) -> p g x", x=320)
                nc.scalar.activation(
                    pt3, dp3[:, :, 0:320], EXP,
                    bias=cmask_t[:, jt:jt + 1], scale=SCALE_H)
                pts[(u, jt)] = pt
                if hp == 0 and s == 0 and "dbg_pt" in t_out:
                    nc.gpsimd.dma_start(out=t_out["dbg_pt"], in_=pt)
                for _ in range(4):
                    if pend:
                        pend.pop(0)()
            while pend:
                pend.pop(0)()
            for fin in finishers:
                fin()

            # attn@V on resident pt tiles + per-partition normalization
            for it in range(JT):
                P = Ppp.tile([128, 512], F32, tag="P", name=f"P{hp}{it}")
                for jt in range(JT):
                    for u in (0, 1):
                        h = 2 * hp + u
                        nc.tensor.matmul(
                            P[:, u * 65:u * 65 + 65],
                            pts[(u, jt)][:, it * 128:(it + 1) * 128],
                            v_tiles[jt][:, h * 65:(h + 1) * 65],
                            start=(jt == 0 and u == 0),
                            stop=(jt == JT - 1 and u == 1),
                        )
                if hp == 0 and it == 0 and "dbg_P" in t_out:
                    dbgp = smallp.tile([128, 512], F32, tag="dbgp", bufs=1,
                                       name="dbgp")
                    nc.vector.tensor_copy(dbgp, P)
                    nc.gpsimd.dma_start(out=t_out["dbg_P"], in_=dbgp)
                rec = smallp.tile([128, 2], F32, tag="rec", bufs=3,
                                  name=f"rec{hp}{it}")
                nc.vector.reciprocal(rec, P[:, 64:130:65])
                aon = aonp.tile([128, 128], BF16, tag="aon",
                                name=f"aon{hp}{it}")
                for u in (0, 1):
                    nc.vector.tensor_scalar_mul(
                        aon[:, u * 64:(u + 1) * 64],
                        P[:, u * 65:u * 65 + 64],
                        rec[:, u:u + 1])
                aon_tiles[hp][it] = aon

        # remaining pooled-attention tiles, PE-dense over the exp drain
        for it_l in range(2, NT):
            ops, fin = pooled_early(it_l)
            for op in ops:
                op()
            fin()

    def phase_tr(trp, aop):
        for hp in range(NT):
            tr = trp.tile([128, JC], BF16, tag="tr", name=f"tr{hp}")
            for k in range(JT):
                nc.tensor.matmul(
                    tr[:, k * 128:(k + 1) * 128],
                    aon_tiles[hp][k][:], ident[:], is_transpose=True)
            ao = aop.tile([128, JC], BF16, tag="ao", name=f"ao{hp}")
            nc.vector.tensor_copy(ao, tr)
            ao_tiles[hp] = ao
            if hp == 0 and "dbg_ao" in t_out:
                nc.gpsimd.dma_start(out=t_out["dbg_ao"], in_=ao)

    def phase_out(wop, otp, opsp):
        for dh in range(2):
            pss = [opsp.tile([128, 512], F32, tag="ops", name=f"ps_o{dh}{i}")
                   for i in range(JT)]
            for ft in range(NT):
                w = wop.tile([128, 512], BF16, tag="wo", name="w_o")
                nc.scalar.dma_start(
                    out=w, in_=t_in["wout"][ft * 128:(ft + 1) * 128,
                                            dh * 512:(dh + 1) * 512])
                for it in range(JT):
                    nc.tensor.matmul(
                        pss[it][:],
                        ao_tiles[ft][:, it * 128:(it + 1) * 128],
                        w[:],
                        start=(ft == 0), stop=False,
                    )
            for it in range(JT):
                nc.tensor.matmul(
                    pss[it][:],
                    fixl[:, it * 128:(it + 1) * 128],
                    fixr[:, dh * 512:(dh + 1) * 512],
                    start=False, stop=True,
                )
                ot = otp.tile([128, 512], F32, tag="ot", bufs=8,
                              name=f"ot{dh}{it}")
                nc.vector.tensor_copy(ot, pss[it])
                eng = nc.sync if it % 2 == 0 else nc.scalar
                eng.dma_start(
                    out=t_out["out"][it * 128:(it + 1) * 128,
                                     dh * 512:(dh + 1) * 512],
                    in_=ot)

    with (
        tc.tile_pool(name="xc", bufs=8) as xcp,
        tc.tile_pool(name="wv", bufs=3) as wvp,
        tc.tile_pool(name="wk2", bufs=2) as wk2p,
        ExitStack() as es_vkc,
    ):
        phase_vkc(xcp, wvp, wk2p, es_vkc)

    with (
        tc.tile_pool(name="pe", bufs=2) as pep,
        tc.tile_pool(name="aon", bufs=40) as aonp,
    ):
        with (
            tc.tile_pool(name="pt", bufs=10) as ptp,
            tc.tile_pool(name="wq", bufs=3) as wqp,
            tc.tile_pool(name="pj", bufs=2, space="PSUM") as projp,
            tc.tile_pool(name="dp", bufs=2, space="PSUM") as dpp,
            tc.tile_pool(name="Pp", bufs=2, space="PSUM") as Ppp,
        ):
            head_loop(ptp, wqp, projp, dpp, Ppp, pep, aonp)

        with tc.tile_pool(name="ao", bufs=8) as aop:
            with tc.tile_pool(name="tr", bufs=2, space="PSUM") as trp:
                phase_tr(trp, aop)

            with (
                tc.tile_pool(name="wo", bufs=3) as wop,
                tc.tile_pool(name="ott", bufs=8) as otp,
                tc.tile_pool(name="ops", bufs=JT, space="PSUM") as opsp,
            ):
                phase_out(wop, otp, opsp)


def build_nc():
    nc = bacc.Bacc("TRN2", target_bir_lowering=False, debug=False,
                   num_devices=8)
    t_in = {
        "xT": nc.dram_tensor("xT", [D, N], F32R, kind="ExternalInput").ap(),
        "xcT": nc.dram_tensor("xcT", [D, JC], F32R, kind="ExternalInput").ap(),
        "wqkp": nc.dram_tensor("wqkp", [D, 2 * D], F32R,
                               kind="ExternalInput").ap(),
        "wv_in": nc.dram_tensor("wv_in", [D, D], F32R,
                                kind="ExternalInput").ap(),
        "wout": nc.dram_tensor("wout", [D, D], BF16,
                               kind="ExternalInput").ap(),
        "cmaskTc": nc.dram_tensor("cmaskTc", [128, JT], F32,
                                  kind="ExternalInput").ap(),
        "fixl_in": nc.dram_tensor("fixl_in", [1, JC], BF16,
                                  kind="ExternalInput").ap(),
        "fixr_in": nc.dram_tensor("fixr_in", [1, N], BF16,
                                  kind="ExternalInput").ap(),
        "ident_in": nc.dram_tensor("ident_in", [128, 128], BF16,
                                   kind="ExternalInput").ap(),
    }
    t_out = {
        "out": nc.dram_tensor("out", [JC, D], F32, kind="ExternalOutput").ap(),
        "attn": nc.dram_tensor("attn", [N, N], F32,
                               kind="ExternalOutput").ap(),
    }
    if os.environ.get("ATTN_DEBUG"):
        t_out["dbg_kc"] = nc.dram_tensor(
            "dbg_kc", [128, JC], F32R, kind="ExternalOutput").ap()
        t_out["dbg_v"] = nc.dram_tensor(
            "dbg_v", [128, 65 * H], BF16, kind="ExternalOutput").ap()
        t_out["dbg_pt"] = nc.dram_tensor(
            "dbg_pt", [128, JC], BF16, kind="ExternalOutput").ap()
        t_out["dbg_P"] = nc.dram_tensor(
            "dbg_P", [128, 512], F32, kind="ExternalOutput").ap()
        t_out["dbg_ao"] = nc.dram_tensor(
            "dbg_ao", [128, JC], BF16, kind="ExternalOutput").ap()
    with tile.TileContext(
            nc, trace_sim=bool(os.environ.get('ATTN_TRACE_SIM'))) as tc:
        with ExitStack() as es:
            _body(nc, tc, es, t_in, t_out)
    nc.compile()
    return nc


def _bf16(a):
    import ml_dtypes
    return np.asarray(a, np.float32).astype(ml_dtypes.bfloat16)


def _host_prep(x, mask, w_qkv, w_out, b_out):
    wq3 = w_qkv.reshape(D, 3, NT, 128)
    wqkp = np.ascontiguousarray(
        np.stack([wq3[:, 0], wq3[:, 1]], axis=2).reshape(D, 2 * D))
    wv = np.ascontiguousarray(w_qkv[:, 2 * D:])
    wout_b = _bf16(w_out)
    ident = _bf16(np.eye(128, dtype=np.float32))
    ones_c = _bf16(np.ones((1, JC), np.float32))
    in_maps, aux = [], []
    for b in range(B):
        m = np.concatenate([[True], mask[b]])               # [N]
        keep = np.nonzero(m)[0]
        nk = len(keep)
        assert nk <= JC, f"keep count {nk} exceeds JC={JC}"
        idx = np.concatenate([keep, np.zeros(JC - nk, np.int64)])
        cm_c = np.where(np.arange(JC) < nk, 0.0, NEG).astype(np.float32)
        xTb = np.ascontiguousarray(x[b].T)                  # [D, N]
        xcT = np.ascontiguousarray(xTb[:, idx])             # [D, JC]
        ybar = (x[b].mean(axis=0) @ wv) @ w_out             # [D]
        in_maps.append({
            "xT": xTb,
            "xcT": xcT,
            "wqkp": wqkp,
            "wv_in": wv,
            "wout": wout_b,
            "cmaskTc": np.ascontiguousarray(cm_c.reshape(JT, 128).T),
            "fixl_in": ones_c,
            "fixr_in": _bf16(b_out.reshape(1, D)),
            "ident_in": ident,
        })
        aux.append((m, keep, nk, (b_out + ybar).astype(np.float32)))
    return in_maps, aux


def kernel(x, mask, w_qkv, w_out, b_out, **run_kw):
    if "nc" not in _CACHED:
        _CACHED["nc"] = build_nc()
    nc = _CACHED["nc"]
    in_maps, aux = _host_prep(
        np.asarray(x, np.float32), np.asarray(mask),
        np.asarray(w_qkv, np.float32), np.asarray(w_out, np.float32),
        np.asarray(b_out, np.float32))
    try:
        res = bass_utils.run_bass_kernel_spmd(
            nc, in_maps, core_ids=list(range(B)), **run_kw)
    except Exception:
        # transient NRT device wedge: retry once
        res = bass_utils.run_bass_kernel_spmd(
            nc, in_maps, core_ids=list(range(B)), **run_kw)
    outs = []
    for b in range(B):
        m, keep, nk, masked_row = aux[b]
        oc = np.asarray(res.results[b]["out"], np.float32)   # [JC, D]
        full = np.empty((N, D), np.float32)
        full[~m] = masked_row
        full[keep] = oc[:nk]
        outs.append(full)
    out = np.stack(outs)
    attn_ = np.stack([res.results[b]["attn"] for b in range(B)])
    _CACHED["last_results"] = res
    return out, attn_
